# revision 34
# baseline (speedup 1.0000x reference)
"""ComputeAlignmentError kernel for 8 TRN2 NeuronCores.

Math: for each batch b, pairwise alignment error
    err[i,j] = || Ep_j (pc_i - bp_j) - Et_j (tc_i - bt_j) + eps ||_2
where Ep/Et are orthonormal frame bases built from pred/true frames and
bp/bt are the frame origins.  Because Ep/Et are rotations, err^2[i,j]
collapses exactly into a rank-18 bilinear form  err^2[i,j] = Y[i] . Z[j]:
    Y[i] = [1, |pc|^2, |tc|^2, pc, tc, vec(pc tc^T)]          (18)
    Z[j] = [z0, 1, 1, -2(bp - R bt - eps sp), -2(bt - R^T bp + eps st),
            -2 vec(R)]                                         (18)
    R_j = Ep_j^T Et_j, sp = sum_k ep_k, st = sum_k et_k,
    z0  = |bp|^2 + |bt|^2 + 3 eps^2 - 2 bp.R bt - 2 eps bp.sp + 2 eps bt.st
The mask folds in for free: Y *= mask_i, Z *= mask_j.

The O(n) feature vectors Y/Z are tiny (2048 x 18 floats) and are computed
on the host in float64, pre-transposed into the exact feature-major SBUF
layout the PE needs (feature slots padded 18 -> 32, pads zeroed, with the
j range split into 4 partition bands of 512 and the Y features replicated
onto all four bands).  The device then only runs the O(n^2) part: one
256KB fp16 feature DMA in, per (i-chunk, band) K=32 float16 matmuls at
distinct PE tile positions (so weight loads overlap prior matmuls),
PSUM -> SBUF evacuation as bf16 (DVE and ACT in parallel, with ACT
always the last reader of each PSUM tile -- see the comment in
_kernel_body), and one 512KB DMA per i-chunk out on alternating HWDGE
rings.  A burst of dummy matmuls during the input DMA wait warms the PE
HAM clock gate.  The final sqrt runs on the host (clamped at 0), which
sidesteps float32r's tiny-negative err^2.

Each core handles one (batch, 512-row i-slice): core c -> batch c//4,
rows [512*(c%4), 512*(c%4+1)).
"""

import sys

import numpy as np

sys.path.insert(0, "/opt/trn_rl_repo")

from contextlib import ExitStack

import concourse.bacc as bacc
import concourse.tile as tile
from concourse import mybir
from concourse.bass_utils import run_bass_kernel_spmd

F32 = mybir.dt.float32
F16 = mybir.dt.float16
BF16 = mybir.dt.bfloat16
EPS = 1e-8  # both EPS_FRAME and EPS_DIST in the reference

B, N = 2, 2048
NCORES = 8
ISLICE = N * B // NCORES  # 512 rows of i per core
NITILE = ISLICE // 128  # 4 i-chunks per core
NF = 18  # feature count K
FPAD = 32  # feature slot padding (pads are zeroed; matmul K=32)
NWARM = 7  # PE HAM warm-up matmuls issued during the input DMA wait

NUM_DEVICES = 1  # no collectives -> compile as single-device program


def _build(nc_holder=[]):
    if nc_holder:
        return nc_holder[0]
    nc = bacc.Bacc(
        "TRN2",
        target_bir_lowering=False,
        debug=False,
        enable_asserts=True,
        num_devices=NUM_DEVICES,
    )
    fz_in = nc.dram_tensor("fz", [128, 1024], F16, kind="ExternalInput").ap()
    out_dram = nc.dram_tensor("out", [ISLICE, N], BF16, kind="ExternalOutput").ap()

    with tile.TileContext(nc) as tc, ExitStack() as ctx:
        _kernel_body(ctx, tc, out_dram, fz_in)

    nc.compile()
    nc_holder.append(nc)
    return nc


def _kernel_body(ctx, tc, out_dram, fz_in):
    nc = tc.nc
    P = 128
    H2 = N // 2
    sb = ctx.enter_context(tc.tile_pool(name="sb", bufs=1))
    outp = ctx.enter_context(tc.tile_pool(name="outp", bufs=4))
    psum = ctx.enter_context(tc.tile_pool(name="psum", bufs=4, space="PSUM"))

    # ---- single input DMA: one completion semaphore covers all bytes -----
    # (two parallel DMAs would each get their own DMAHW lane, and the Tile
    # scheduler only threads one of them into the matmuls' waits; the DMA
    # wall time is dominated by the fixed completion latency anyway, and
    # partition-subset DMAs run at a fraction of the fabric rate).
    # Features travel as fp16 (10-bit mantissa): halves the transfer vs
    # f32r while the output's bf16 quantization still dominates the error.
    FZ = sb.tile([P, 1024], F16, tag="FZ")
    nc.sync.dma_start(out=FZ[:], in_=fz_in[:])
    ZT = FZ[:, 0:512]
    YT = FZ[:, 512:1024]

    # ---- PE clock warm-up during the DMA wait ----------------------------
    # The HAM gate halves the PE clock until it sees ~3.4us of sustained
    # activity.  Dummy fp32 matmuls run while the feature DMA is in flight
    # so the real matmuls start at full rate.  Any initialized tile works
    # as the operand (results are discarded), so a single memset replaces
    # make_identity's memset+iota+affine_select chain.  They write into
    # it0's pmA bank, which the first real matmul (start=True) clears and
    # overwrites -- no extra reader, no keep output needed.
    ident = sb.tile([P, P], F32, tag="ident")
    nc.gpsimd.memset(ident[:], 1.0)
    pms = []
    for it in range(NITILE):
        pms.append(
            (
                psum.tile([P, H2], F32, tag="mm", name=f"pmA{it}"),
                psum.tile([P, H2], F32, tag="mm", name=f"pmB{it}"),
            )
        )
    for k in range(NWARM):
        nc.tensor.matmul(
            pms[0][0][:, 0:P],
            ident[:],
            ident[:],
            start=(k == 0),
            stop=(k == NWARM - 1),
        )

    # ---- main: matmul (K=32, float32r) + bf16 copy + DMA out -------------
    # Band cl (partitions 32cl:32cl+32) holds Z features of the contiguous
    # j range [512cl, 512(cl+1)) and a replica of the Y features; the four
    # bands map to distinct PE tile positions so each matmul's weight load
    # overlaps the previous matmul.
    #
    # PSUM evacuation: the Tile scheduler encodes cross-engine waits for
    # ACT readers but elides DVE-reader waits based on modeled timing
    # (CoreSim models the DVE f32->bf16 cast ~2x faster than hardware), so
    # a consumer keyed on the ACT semaphore can race a still-running DVE
    # read.  The sound structure: ACT is the real-time LAST reader of both
    # PSUM tiles -- DVE copies pmA[:, 0:960] (starts after matmul cl1,
    # ends early), ACT copies all of pmB and then a 64-column tail of pmA
    # (ends ~0.5us after DVE).  Slot-reuse matmuls and the output DMA then
    # wait on ACT sems, which really do cover the DVE read.
    TAIL = 64
    for it in range(NITILE):
        ot = outp.tile([P, N], BF16, tag="ot")
        pmA, pmB = pms[it]
        for cl in range(4):
            rg = 32 * cl
            pm = pmA if cl < 2 else pmB
            nc.tensor.matmul(
                pm[:, 512 * (cl % 2) : 512 * (cl % 2 + 1)],
                YT[rg : rg + FPAD, it * P : (it + 1) * P],
                ZT[rg : rg + FPAD, 0:512],
                start=True,
                stop=True,
                tile_position=(rg, 0),
            )
        nc.vector.tensor_copy(ot[:, 0 : H2 - TAIL], pmA[:, 0 : H2 - TAIL])
        nc.scalar.copy(ot[:, H2:N], pmB[:])
        nc.scalar.copy(ot[:, H2 - TAIL : H2], pmA[:, H2 - TAIL : H2])
        rows = out_dram[it * P : (it + 1) * P, :]
        # alternate the two HWDGE rings (sync / scalar) so two output DMAs
        # stream in parallel instead of serializing on one ring
        eng = nc.sync if it % 2 == 0 else nc.scalar
        eng.dma_start(out=rows, in_=ot[:])


def _l2n(t):
    n = np.linalg.norm(t, axis=-1, keepdims=True)
    return t / np.maximum(n, EPS)


def _frame_basis(frames):
    # frames: [n, 3(xyz), 3(points a,b,c)]
    a, b, c = frames[..., 0], frames[..., 1], frames[..., 2]
    w1 = _l2n(a - b)
    w2 = _l2n(c - b)
    e1 = _l2n(w1 + w2)
    e2 = _l2n(w2 - w1)
    e3 = np.cross(e1, e2)
    E = np.stack((e1, e2, e3), axis=-2)  # [n, 3(basis k), 3(xyz)]
    return b, E


def _features(pc, tc, pf, tf, mk):
    """Per-batch Y [n,18] / Z [n,18] feature vectors (float64 in, float64 out)."""
    n = pc.shape[0]
    bp, Ep = _frame_basis(pf)
    bt, Et = _frame_basis(tf)
    R = np.einsum("nka,nkb->nab", Ep, Et)
    sp = Ep.sum(axis=1)
    st = Et.sum(axis=1)
    Rbt = np.einsum("nab,nb->na", R, bt)
    Rtbp = np.einsum("nab,na->nb", R, bp)
    z0 = (
        (bp * bp).sum(-1)
        + (bt * bt).sum(-1)
        + 3.0 * EPS * EPS
        - 2.0 * (bp * Rbt).sum(-1)
        - 2.0 * EPS * (sp * bp).sum(-1)
        + 2.0 * EPS * (st * bt).sum(-1)
    )
    ones = np.ones((n, 1))
    Z = np.concatenate(
        [
            z0[:, None],
            ones,
            ones,
            -2.0 * bp + 2.0 * Rbt + 2.0 * EPS * sp,
            -2.0 * bt + 2.0 * Rtbp - 2.0 * EPS * st,
            -2.0 * R.reshape(n, 9),
        ],
        axis=1,
    )
    Y = np.concatenate(
        [
            ones,
            (pc * pc).sum(-1)[:, None],
            (tc * tc).sum(-1)[:, None],
            pc,
            tc,
            (pc[:, :, None] * tc[:, None, :]).reshape(n, 9),
        ],
        axis=1,
    )
    Z *= mk[:, None]
    Y *= mk[:, None]
    return Y, Z


def _shard_inputs(pred_coords, true_coords, pred_frames, true_frames, mask):
    """Host-side O(n) feature build into per-core feature-major layouts."""
    pc = np.asarray(pred_coords, np.float64)
    tc = np.asarray(true_coords, np.float64)
    pf = np.asarray(pred_frames, np.float64)
    tf = np.asarray(true_frames, np.float64)
    mk = np.asarray(mask).astype(np.float64)

    in_maps = []
    for b in range(B):
        Y, Z = _features(pc[b], tc[b], pf[b], tf[b], mk[b])
        # ZT[32cl+f, jj] = Z[512cl+jj, f]; shared by the batch's 4 cores
        Zp = np.zeros((4, FPAD, 512), np.float16)
        Zp[:, :NF, :] = Z.reshape(4, 512, NF).transpose(0, 2, 1)
        ZT = np.ascontiguousarray(Zp.reshape(128, 512))
        for s in range(NCORES // B):
            i0 = s * ISLICE
            # YT[32cl+f, ii] = Y[i0+ii, f], replicated on all 4 bands
            Yp = np.zeros((4, FPAD, 512), np.float16)
            Yp[:, :NF, :] = Y[i0 : i0 + ISLICE].T[None]
            YT = Yp.reshape(128, 512)
            in_maps.append(
                {"fz": np.ascontiguousarray(np.concatenate([ZT, YT], axis=1))}
            )
    return in_maps


def kernel(pred_coords, true_coords, pred_frames, true_frames, mask, _res=[]):
    nc = _build()
    in_maps = _shard_inputs(pred_coords, true_coords, pred_frames, true_frames, mask)
    res = run_bass_kernel_spmd(nc, in_maps, list(range(NCORES)))
    _res.clear()
    _res.append(res)
    out = np.empty((B, N, N), np.float32)
    for core in range(NCORES):
        b = core // (NCORES // B)
        i0 = (core % (NCORES // B)) * ISLICE
        err2 = res.results[core]["out"].astype(np.float32)
        out[b, i0 : i0 + ISLICE, :] = np.sqrt(np.maximum(err2, 0.0))
    return out


if __name__ == "__main__":
    rng = np.random.default_rng(0)
    ins = {
        "pred_coords": rng.standard_normal((B, N, 3)).astype(np.float32),
        "true_coords": rng.standard_normal((B, N, 3)).astype(np.float32),
        "pred_frames": rng.standard_normal((B, N, 3, 3)).astype(np.float32),
        "true_frames": rng.standard_normal((B, N, 3, 3)).astype(np.float32),
        "mask": np.ones((B, N), bool),
    }
    out = kernel(**ins)
    print("out", out.shape, out.dtype, float(np.abs(out).max()))


# revision 35
# speedup vs baseline: 1.0317x; 1.0317x over previous
"""ComputeAlignmentError kernel for 8 TRN2 NeuronCores.

Math: for each batch b, pairwise alignment error
    err[i,j] = || Ep_j (pc_i - bp_j) - Et_j (tc_i - bt_j) + eps ||_2
where Ep/Et are orthonormal frame bases built from pred/true frames and
bp/bt are the frame origins.  Because Ep/Et are rotations, err^2[i,j]
collapses exactly into a rank-18 bilinear form  err^2[i,j] = Y[i] . Z[j]:
    Y[i] = [1, |pc|^2, |tc|^2, pc, tc, vec(pc tc^T)]          (18)
    Z[j] = [z0, 1, 1, -2(bp - R bt - eps sp), -2(bt - R^T bp + eps st),
            -2 vec(R)]                                         (18)
    R_j = Ep_j^T Et_j, sp = sum_k ep_k, st = sum_k et_k,
    z0  = |bp|^2 + |bt|^2 + 3 eps^2 - 2 bp.R bt - 2 eps bp.sp + 2 eps bt.st
The mask folds in for free: Y *= mask_i, Z *= mask_j.

The O(n) feature vectors Y/Z are tiny (2048 x 18 floats) and are computed
on the host in float64, pre-transposed into the exact feature-major SBUF
layout the PE needs (feature slots padded 18 -> 32, pads zeroed, with the
j range split into 4 partition bands of 512 and the Y features replicated
onto all four bands).  The device then only runs the O(n^2) part: one
256KB fp16 feature DMA in, per (i-chunk, band) K=32 float16 matmuls at
distinct PE tile positions (so weight loads overlap prior matmuls),
PSUM -> SBUF evacuation as bf16 (DVE and ACT in parallel, with ACT
always the last reader of each PSUM tile -- see the comment in
_kernel_body), and one 512KB DMA per i-chunk out on alternating HWDGE
rings.  A burst of dummy matmuls during the input DMA wait warms the PE
HAM clock gate.  The final sqrt runs on the host (clamped at 0), which
sidesteps the matmul's tiny-negative err^2 rounding.

Each core handles one (batch, 512-row i-slice): core c -> batch c//4,
rows [512*(c%4), 512*(c%4+1)).
"""

import sys

import numpy as np

sys.path.insert(0, "/opt/trn_rl_repo")

from contextlib import ExitStack

import concourse.bacc as bacc
import concourse.tile as tile
from concourse import mybir
from concourse.bass_utils import run_bass_kernel_spmd

F32 = mybir.dt.float32
F16 = mybir.dt.float16
BF16 = mybir.dt.bfloat16
EPS = 1e-8  # both EPS_FRAME and EPS_DIST in the reference

B, N = 2, 2048
NCORES = 8
ISLICE = N * B // NCORES  # 512 rows of i per core
NITILE = ISLICE // 128  # 4 i-chunks per core
NF = 18  # feature count K
FPAD = 32  # feature slot padding (pads are zeroed; matmul K=32)
NWARM = 7  # PE HAM warm-up matmuls issued during the input DMA wait

NUM_DEVICES = 1  # no collectives -> compile as single-device program


def _build(nc_holder=[]):
    if nc_holder:
        return nc_holder[0]
    nc = bacc.Bacc(
        "TRN2",
        target_bir_lowering=False,
        debug=False,
        enable_asserts=True,
        num_devices=NUM_DEVICES,
    )
    fz_in = nc.dram_tensor("fz", [128, 1024], F16, kind="ExternalInput").ap()
    out_dram = nc.dram_tensor("out", [ISLICE, N], BF16, kind="ExternalOutput").ap()

    with tile.TileContext(nc) as tc, ExitStack() as ctx:
        _kernel_body(ctx, tc, out_dram, fz_in)

    nc.compile()
    nc_holder.append(nc)
    return nc


def _kernel_body(ctx, tc, out_dram, fz_in):
    nc = tc.nc
    P = 128
    H2 = N // 2
    sb = ctx.enter_context(tc.tile_pool(name="sb", bufs=1))
    outp = ctx.enter_context(tc.tile_pool(name="outp", bufs=4))
    psum = ctx.enter_context(tc.tile_pool(name="psum", bufs=4, space="PSUM"))

    # ---- single input DMA: one completion semaphore covers all bytes -----
    # (two parallel DMAs would each get their own DMAHW lane, and the Tile
    # scheduler only threads one of them into the matmuls' waits; the DMA
    # wall time is dominated by the fixed completion latency anyway, and
    # partition-subset DMAs run at a fraction of the fabric rate).
    # Features travel as fp16 (10-bit mantissa): halves the transfer vs
    # f32r while the output's bf16 quantization still dominates the error.
    FZ = sb.tile([P, 1024], F16, tag="FZ")
    nc.sync.dma_start(out=FZ[:], in_=fz_in[:])
    ZT = FZ[:, 0:512]
    YT = FZ[:, 512:1024]

    # ---- PE clock warm-up during the DMA wait ----------------------------
    # The HAM gate halves the PE clock until it sees ~3.4us of sustained
    # activity.  Dummy fp32 matmuls run while the feature DMA is in flight
    # so the real matmuls start at full rate.  Any initialized tile works
    # as the operand (results are discarded), so a single memset replaces
    # make_identity's memset+iota+affine_select chain.  They write into
    # it0's pmA bank, which the first real matmul (start=True) clears and
    # overwrites -- no extra reader, no keep output needed.
    ident = sb.tile([P, P], F32, tag="ident")
    nc.gpsimd.memset(ident[:], 1.0)
    pms = []
    for it in range(NITILE):
        pms.append(
            (
                psum.tile([P, H2], F32, tag="mm", name=f"pmA{it}"),
                psum.tile([P, H2], F32, tag="mm", name=f"pmB{it}"),
            )
        )
    for k in range(NWARM):
        nc.tensor.matmul(
            pms[0][0][:, 0:P],
            ident[:],
            ident[:],
            start=(k == 0),
            stop=(k == NWARM - 1),
        )

    # ---- main: matmul (K=32, float16) + bf16 copy + DMA out --------------
    # Band cl (partitions 32cl:32cl+32) holds Z features of the contiguous
    # j range [512cl, 512(cl+1)) and a replica of the Y features; the four
    # bands map to distinct PE tile positions so each matmul's weight load
    # overlaps the previous matmul.
    #
    # PSUM evacuation: the Tile scheduler encodes cross-engine waits for
    # ACT readers but elides DVE-reader waits based on modeled timing
    # (CoreSim models the DVE f32->bf16 cast ~2x faster than hardware), so
    # a consumer keyed on the ACT semaphore can race a still-running DVE
    # read.  The sound structure: ACT is the real-time LAST reader of both
    # PSUM tiles -- DVE copies pmA[:, 0:960] (starts after matmul cl1,
    # ends early), ACT copies all of pmB and then a 64-column tail of pmA
    # (ends ~0.5us after DVE).  Slot-reuse matmuls and the output DMA then
    # wait on ACT sems, which really do cover the DVE read.
    TAIL = 64
    for it in range(NITILE):
        ot = outp.tile([P, N], BF16, tag="ot")
        pmA, pmB = pms[it]
        for cl in range(4):
            rg = 32 * cl
            pm = pmA if cl < 2 else pmB
            nc.tensor.matmul(
                pm[:, 512 * (cl % 2) : 512 * (cl % 2 + 1)],
                YT[rg : rg + FPAD, it * P : (it + 1) * P],
                ZT[rg : rg + FPAD, 0:512],
                start=True,
                stop=True,
                tile_position=(rg, 0),
            )
        nc.vector.tensor_copy(ot[:, 0 : H2 - TAIL], pmA[:, 0 : H2 - TAIL])
        nc.scalar.copy(ot[:, H2:N], pmB[:])
        nc.scalar.copy(ot[:, H2 - TAIL : H2], pmA[:, H2 - TAIL : H2])
        rows = out_dram[it * P : (it + 1) * P, :]
        # alternate the two HWDGE rings (sync / scalar) so two output DMAs
        # stream in parallel instead of serializing on one ring
        eng = nc.sync if it % 2 == 0 else nc.scalar
        eng.dma_start(out=rows, in_=ot[:])


def _l2n(t):
    n = np.linalg.norm(t, axis=-1, keepdims=True)
    return t / np.maximum(n, EPS)


def _frame_basis(frames):
    # frames: [n, 3(xyz), 3(points a,b,c)]
    a, b, c = frames[..., 0], frames[..., 1], frames[..., 2]
    w1 = _l2n(a - b)
    w2 = _l2n(c - b)
    e1 = _l2n(w1 + w2)
    e2 = _l2n(w2 - w1)
    e3 = np.cross(e1, e2)
    E = np.stack((e1, e2, e3), axis=-2)  # [n, 3(basis k), 3(xyz)]
    return b, E


def _features(pc, tc, pf, tf, mk):
    """Per-batch Y [n,18] / Z [n,18] feature vectors (float64 in, float64 out)."""
    n = pc.shape[0]
    bp, Ep = _frame_basis(pf)
    bt, Et = _frame_basis(tf)
    R = np.einsum("nka,nkb->nab", Ep, Et)
    sp = Ep.sum(axis=1)
    st = Et.sum(axis=1)
    Rbt = np.einsum("nab,nb->na", R, bt)
    Rtbp = np.einsum("nab,na->nb", R, bp)
    z0 = (
        (bp * bp).sum(-1)
        + (bt * bt).sum(-1)
        + 3.0 * EPS * EPS
        - 2.0 * (bp * Rbt).sum(-1)
        - 2.0 * EPS * (sp * bp).sum(-1)
        + 2.0 * EPS * (st * bt).sum(-1)
    )
    ones = np.ones((n, 1))
    Z = np.concatenate(
        [
            z0[:, None],
            ones,
            ones,
            -2.0 * bp + 2.0 * Rbt + 2.0 * EPS * sp,
            -2.0 * bt + 2.0 * Rtbp - 2.0 * EPS * st,
            -2.0 * R.reshape(n, 9),
        ],
        axis=1,
    )
    Y = np.concatenate(
        [
            ones,
            (pc * pc).sum(-1)[:, None],
            (tc * tc).sum(-1)[:, None],
            pc,
            tc,
            (pc[:, :, None] * tc[:, None, :]).reshape(n, 9),
        ],
        axis=1,
    )
    Z *= mk[:, None]
    Y *= mk[:, None]
    return Y, Z


def _shard_inputs(pred_coords, true_coords, pred_frames, true_frames, mask):
    """Host-side O(n) feature build into per-core feature-major layouts."""
    pc = np.asarray(pred_coords, np.float64)
    tc = np.asarray(true_coords, np.float64)
    pf = np.asarray(pred_frames, np.float64)
    tf = np.asarray(true_frames, np.float64)
    mk = np.asarray(mask).astype(np.float64)

    in_maps = []
    for b in range(B):
        Y, Z = _features(pc[b], tc[b], pf[b], tf[b], mk[b])
        # ZT[32cl+f, jj] = Z[512cl+jj, f]; shared by the batch's 4 cores
        Zp = np.zeros((4, FPAD, 512), np.float16)
        Zp[:, :NF, :] = Z.reshape(4, 512, NF).transpose(0, 2, 1)
        ZT = np.ascontiguousarray(Zp.reshape(128, 512))
        for s in range(NCORES // B):
            i0 = s * ISLICE
            # YT[32cl+f, ii] = Y[i0+ii, f], replicated on all 4 bands
            Yp = np.zeros((4, FPAD, 512), np.float16)
            Yp[:, :NF, :] = Y[i0 : i0 + ISLICE].T[None]
            YT = Yp.reshape(128, 512)
            in_maps.append(
                {"fz": np.ascontiguousarray(np.concatenate([ZT, YT], axis=1))}
            )
    return in_maps


def kernel(pred_coords, true_coords, pred_frames, true_frames, mask, _res=[]):
    nc = _build()
    in_maps = _shard_inputs(pred_coords, true_coords, pred_frames, true_frames, mask)
    res = run_bass_kernel_spmd(nc, in_maps, list(range(NCORES)))
    _res.clear()
    _res.append(res)
    out = np.empty((B, N, N), np.float32)
    for core in range(NCORES):
        b = core // (NCORES // B)
        i0 = (core % (NCORES // B)) * ISLICE
        err2 = res.results[core]["out"].astype(np.float32)
        out[b, i0 : i0 + ISLICE, :] = np.sqrt(np.maximum(err2, 0.0))
    return out


if __name__ == "__main__":
    rng = np.random.default_rng(0)
    ins = {
        "pred_coords": rng.standard_normal((B, N, 3)).astype(np.float32),
        "true_coords": rng.standard_normal((B, N, 3)).astype(np.float32),
        "pred_frames": rng.standard_normal((B, N, 3, 3)).astype(np.float32),
        "true_frames": rng.standard_normal((B, N, 3, 3)).astype(np.float32),
        "mask": np.ones((B, N), bool),
    }
    out = kernel(**ins)
    print("out", out.shape, out.dtype, float(np.abs(out).max()))
